# revision 4
# baseline (speedup 1.0000x reference)
"""Trainium2 Bass kernel for DilatedSpatialAttention, v3.

Problem (hardcoded): B=16, H=W=32, C=256, heads=8, head_dim=32,
depthwise 3x3 conv with dilation 2 (SAME) applied to key and value,
then standard softmax attention per (batch, head) over S=H*W=1024.

Sharding: data-parallel over batch across 8 cores (2 batches/core).

v3 design (cost-model verified; DVE was the v2 bottleneck at ~195us/core
vs ACT 146 / PE 128):
- Inputs are cast f32->bf16 by a SWDGE DMA into an HBM staging buffer,
  then transpose-DMA'd (xbar, 2-byte dtype) directly into c-major SBUF
  tiles. This removes all 48 input PE transposes and 48 DVE PSUM copies
  per batch that v2 paid.
- The depthwise conv runs on the Tensor engine as 9 accumulating
  matmuls per [128, 512] output block with diagonal per-channel weight
  matrices. Border taps are clipped (partial-region accumulation
  reproduces SAME zero padding exactly), so no padded layout, no
  memsets. Evacuation (PSUM -> SBUF bf16) fuses the conv bias add on
  the DVE (tensor_scalar add).
- v-path is bf16 end to end (vc bf16, vaug transposes via a bf16
  identity), halving those transpose+copy costs.
- Softmax exp can be partially offloaded from ACT (1 elem/cycle/lane)
  to the DVE via a Schraudolph bf16 exp (int16(x*a+b) bitcast to bf16,
  ~1.8% rms elementwise, mean ~0): NAPX of the 8 kt tiles per unit.

Per-core per-batch dataflow:
  1. gpsimd cast-DMA q/k/v f32 [1024, 256] -> HBM bf16 staging.
  2. sync transpose-DMA staging -> qc[half] / kin/vin [128, 1024] bf16.
  3. conv on PE: per (tensor, half, 512-block): 9 diag matmuls
     accumulate in PSUM; DVE evacuates + bias -> kc/vc bf16.
  4. scoresT[k, q] = Kc^T Qc via row-tiled (K=32) matmuls, 2 heads
     packed per unit, PSUM fp32.
  5. P = exp(scale * scoresT): ACT (exact) or DVE (Schraudolph) per kt.
  6. outT[d, q] (+ row 32 = softmax denom) = [V|1]^T P over k.
  7. PE-transpose outT back, normalize with per-partition reciprocal
     broadcast on VectorE, assemble output rows, DMA out.
"""

import math as _math

import numpy as np

B, H, W, C = 16, 32, 32, 256
HEADS = 8
HD = C // HEADS            # 32
KSZ, DIL = 3, 2
SCALE = float(HD) ** -0.5
NCORES = 8
BPC = B // NCORES          # batches per core
S = H * W                  # 1024
NKT = S // 128             # 8 k/s tiles
AV_DEFER = 2               # units between scores and their AV emission
URG_U = 2                  # units that absorb the current batch's v-path
NAPX = 0                   # kt tiles per unit whose exp runs on DVE
EXPA = SCALE * 128.0 / _math.log(2.0)   # Schraudolph scale (folds SCALE)
EXPB = 16256.0 - 8.0                    # 127*128 + rounding centering

_CACHE = {}


def _build(nc, tile, bass, mybir, repeat=None, parts="all"):
    from contextlib import ExitStack
    from concourse.masks import make_identity

    f32 = mybir.dt.float32
    bf16 = mybir.dt.bfloat16
    i16 = mybir.dt.int16

    q_d = nc.dram_tensor("query", [BPC, S, C], f32, kind="ExternalInput")
    k_d = nc.dram_tensor("key_in", [BPC, S, C], f32, kind="ExternalInput")
    v_d = nc.dram_tensor("value", [BPC, S, C], f32, kind="ExternalInput")
    ck_d = nc.dram_tensor("conv_kernel", [KSZ * KSZ, C], f32, kind="ExternalInput")
    cb_d = nc.dram_tensor("conv_bias", [C], f32, kind="ExternalInput")
    out_d = nc.dram_tensor("out", [BPC, S, C], f32, kind="ExternalOutput")
    # bf16 staging in HBM for the xbar transpose loads
    stage = {
        "q": nc.dram_tensor("stage_q", [BPC, S, C], bf16, kind="Internal"),
        "k": nc.dram_tensor("stage_k", [BPC, S, C], bf16, kind="Internal"),
        "v": nc.dram_tensor("stage_v", [BPC, S, C], bf16, kind="Internal"),
    }
    dram_in = {"q": q_d, "k": k_d, "v": v_d}

    with ExitStack() as ctx:
        tc = ctx.enter_context(tile.TileContext(nc))
        const = ctx.enter_context(tc.tile_pool(name="const", bufs=1))
        qc_p = ctx.enter_context(tc.tile_pool(name="qc", bufs=4))
        cin_p = ctx.enter_context(tc.tile_pool(name="cin", bufs=8))
        kc_p = ctx.enter_context(tc.tile_pool(name="kc", bufs=4))
        vc_p = ctx.enter_context(tc.tile_pool(name="vc", bufs=4))
        vaug_p = ctx.enter_context(tc.tile_pool(name="vaug", bufs=16))
        p_p = ctx.enter_context(tc.tile_pool(name="pp", bufs=40))
        ot_p = ctx.enter_context(tc.tile_pool(name="ot", bufs=4))
        orow_p = ctx.enter_context(tc.tile_pool(name="orow", bufs=16))
        small_p = ctx.enter_context(tc.tile_pool(name="small", bufs=8))
        # PSUM budget (8 banks): trans 2 + scores 2x2 + accum 2 + conv 1
        ppp = ctx.enter_context(tc.tile_pool(name="ppp", bufs=1, space="PSUM"))
        sc_p = ctx.enter_context(tc.tile_pool(name="scp", bufs=2, space="PSUM"))
        acc_p = ctx.enter_context(tc.tile_pool(name="accp", bufs=2, space="PSUM"))
        conv_p = ctx.enter_context(tc.tile_pool(name="convp", bufs=1, space="PSUM"))

        # ---- constants ----
        ident = const.tile([128, 128], f32)
        make_identity(nc, ident[:])
        identb = const.tile([128, 128], bf16)
        nc.vector.tensor_copy(out=identb[:], in_=ident[:])

        # conv weights as per-partition scalars: wcol[c, half, tap]
        wcol = const.tile([128, 2, KSZ * KSZ], f32)
        for half in range(2):
            nc.gpsimd.dma_start(
                out=wcol[:, half],
                in_=bass.AP(ck_d, half * 128, [[1, 128], [C, KSZ * KSZ]]),
            )
        bias_c = const.tile([128, 2], f32)
        for half in range(2):
            nc.gpsimd.dma_start(
                out=bias_c[:, half:half + 1],
                in_=bass.AP(cb_d, half * 128, [[1, 128], [1, 1]]),
            )
        # diagonal weight matrices for the PE conv: diagw[c, half, tap, c']
        diagw = const.tile([128, 2, KSZ * KSZ, 128], bf16)
        for half in range(2):
            for tap in range(KSZ * KSZ):
                nc.vector.tensor_scalar(
                    out=diagw[:, half, tap], in0=ident[:],
                    scalar1=wcol[:, half, tap:tap + 1], scalar2=None,
                    op0=mybir.AluOpType.mult)

        rep_ctx = tc.For_i(0, repeat, 1) if repeat else None
        if rep_ctx is not None:
            ctx.enter_context(rep_ctx)

        state = {}

        def prep_chunks(b):
            """Emit-able closures for batch b's prep; fills state[b]."""
            qc = [qc_p.tile([128, S], bf16, tag="qc", name="qc") for _ in range(2)]
            kc = [kc_p.tile([128, S], bf16, tag="kc", name="kc") for _ in range(2)]
            vc = [vc_p.tile([128, S], bf16, tag="vc", name="vc") for _ in range(2)]
            cin = {(t, h): cin_p.tile([128, S], bf16, tag=f"cin{t}{h}",
                                      name="cin")
                   for t in ("k", "v") for h in range(2)}
            vaug = [vaug_p.tile([128, HEADS * (HD + 1)], bf16, tag="va",
                                name="va") for _ in range(NKT)]
            state[b] = (qc, kc, vaug)

            def mk_cast(t):
                def go():
                    nc.gpsimd.dma_start(out=stage[t][b], in_=dram_in[t][b])
                return go

            def mk_tdma(t, half):
                def go():
                    if t == "q":
                        dst = qc[half][:]
                    else:
                        dst = cin[(t, half)][:]
                    nc.sync.dma_start(
                        out=dst,
                        in_=stage[t][b, :, half * 128:half * 128 + 128],
                        transpose=True)
                return go

            def mk_conv(t, half, ib):
                # 9 accumulating diag matmuls; border taps clipped so the
                # skipped contributions reproduce SAME zero padding.
                def go():
                    cin3 = cin[(t, half)][:].rearrange(
                        "p (y x) -> p y x", x=W)
                    cp = conv_p.tile([128, 16, W], f32, tag="cv", name="cv")
                    y0, y1 = 16 * ib, 16 * ib + 16
                    taps = [4] + [tp for tp in range(9) if tp != 4]
                    for i, tap in enumerate(taps):
                        dh, dw = divmod(tap, KSZ)
                        dy, dx = DIL * (dh - 1), DIL * (dw - 1)
                        ylo, yhi = max(y0, -dy), min(y1, H - dy)
                        xlo, xhi = max(0, -dx), min(W, W - dx)
                        nc.tensor.matmul(
                            out=cp[:, ylo - y0:yhi - y0, xlo:xhi],
                            lhsT=diagw[:, half, tap],
                            rhs=cin3[:, ylo + dy:yhi + dy, xlo + dx:xhi + dx],
                            start=(i == 0), stop=(i == len(taps) - 1),
                            skip_group_check=True)
                    dstt = kc[half] if t == "k" else vc[half]
                    nc.vector.tensor_scalar(
                        out=dstt[:, ib * 512:(ib + 1) * 512],
                        in0=cp[:].rearrange("p y x -> p (y x)"),
                        scalar1=bias_c[:, half:half + 1], scalar2=None,
                        op0=mybir.AluOpType.add)
                return go

            def mk_vaug(kt):
                def go():
                    va3 = vaug[kt][:].rearrange("p (h x) -> p h x", x=HD + 1)
                    nc.vector.memset(va3[:, :, HD:HD + 1], 1.0)
                    for half in range(2):
                        ptb = ppp.tile([128, 128], bf16, tag="pp", name="ptb")
                        nc.tensor.transpose(
                            ptb[:], vc[half][:, kt * 128:(kt + 1) * 128],
                            identb[:])
                        nc.vector.tensor_copy(
                            out=va3[:, 4 * half:4 * half + 4, 0:HD],
                            in_=ptb[:].rearrange("p (h d) -> p h d", d=HD))
                return go

            dmas = [mk_cast("k"), mk_cast("q"), mk_cast("v"),
                    mk_tdma("k", 0), mk_tdma("k", 1),
                    mk_tdma("q", 0), mk_tdma("q", 1),
                    mk_tdma("v", 0), mk_tdma("v", 1)]
            # prefix: everything scores need (kc ready)
            prefix = [mk_conv("k", 0, 0), mk_conv("k", 0, 1),
                      mk_conv("k", 1, 0), mk_conv("k", 1, 1)]
            # pending: the v path, consumed by this batch's AVs
            pending = [mk_conv("v", 0, 0), mk_conv("v", 0, 1),
                       mk_conv("v", 1, 0), mk_conv("v", 1, 1)]
            pending += [mk_vaug(kt) for kt in range(NKT)]
            return dmas, prefix, pending

        def attn_units(b):
            qc, kc, vaug = state[b]
            units = []

            def mk_unit(pair, qb):
                half, hl = divmod(pair, 2)
                q0 = qb * 512

                def go(fillers=(), pre_out=None, pre_av=None):
                    fillers = list(fillers)
                    n_f = len(fillers)
                    ptiles = []
                    for kt in range(NKT):
                        # evenly drain ALL assigned fillers across the kts
                        while len(fillers) > n_f * (NKT - 1 - kt) // NKT:
                            fillers.pop(0)()
                        if kt == 2 and pre_out is not None:
                            pre_out()
                        sc = sc_p.tile([128, 2, 512], f32, tag="sc", name="sc")
                        for j in range(2):
                            base = 64 * hl + 32 * j
                            nc.tensor.matmul(
                                out=sc[:, j, :],
                                lhsT=kc[half][base:base + 32,
                                              kt * 128:(kt + 1) * 128],
                                rhs=qc[half][base:base + 32, q0:q0 + 512],
                                start=True, stop=True,
                                tile_position=(base, 0))
                        p = p_p.tile([128, 2, 512], bf16, tag="p", name="p")
                        if kt >= NKT - NAPX:
                            nc.vector.tensor_scalar(
                                out=p[:].bitcast(i16), in0=sc[:],
                                scalar1=EXPA, scalar2=EXPB,
                                op0=mybir.AluOpType.mult,
                                op1=mybir.AluOpType.add)
                        else:
                            nc.scalar.activation(
                                out=p[:], in_=sc[:],
                                func=mybir.ActivationFunctionType.Exp,
                                scale=SCALE)
                        ptiles.append(p)
                    if pre_av is not None:
                        pre_av()
                    return mk_av(ptiles, pair, qb)
                return go

            def mk_av(ptiles, pair, qb):
                half, hl = divmod(pair, 2)

                def av():
                    acc = acc_p.tile([128, 512], f32, tag="acc", name="acc")
                    for kt in range(NKT):
                        for j in range(2):
                            hglob = half * 4 + hl * 2 + j
                            # j==0 widens lhsT to 64 cols so acc rows 33:64
                            # are written (defined junk) -- the later ot copy
                            # of rows 0:97 must not read uninitialized PSUM.
                            w_l = 64 if j == 0 else HD + 1
                            nc.tensor.matmul(
                                out=acc[64 * j:64 * j + w_l, :],
                                lhsT=vaug[kt][:, (HD + 1) * hglob:
                                              (HD + 1) * hglob + w_l],
                                rhs=ptiles[kt][:, j, :],
                                start=(kt == 0), stop=(kt == NKT - 1))
                    h0 = half * 4 + hl * 2
                    ot = ot_p.tile([128, 512], f32, tag="ot", name="ot")
                    nc.vector.tensor_copy(out=ot[0:97, :], in_=acc[0:97, :])
                    def flush_out():
                        for u in range(4):
                          tp = ppp.tile([128, 512], f32, tag="pp", name="pt")
                          nc.tensor.transpose(
                              tp[:, 0:97], ot[0:97, u * 128:(u + 1) * 128],
                              ident[0:97, 0:97])
                          rc = small_p.tile([128, 2], f32, tag="rc", name="rc")
                          sums = bass.AP(tp.tensor, tp.offset + HD,
                                         [tp.ap[0], [64, 2]])
                          nc.vector.reciprocal(rc[:], sums)
                          otile = orow_p.tile([128, 2 * HD], f32, tag="orow",
                                              name="orow")
                          # otile[:, 32j+d] = tp[:, 64j+d] * rc[:, j]
                          src = bass.AP(tp.tensor, tp.offset,
                                        [tp.ap[0], [64, 2], [1, HD]])
                          rcb = bass.AP(rc.tensor, rc.offset,
                                        [rc.ap[0], [1, 2], [0, HD]])
                          nc.vector.tensor_tensor(
                              out=otile[:].rearrange("p (j d) -> p j d", d=HD),
                              in0=src, in1=rcb, op=mybir.AluOpType.mult)
                          nc.sync.dma_start(
                              out=out_d[b, (qb * 4 + u) * 128:
                                        (qb * 4 + u) * 128 + 128,
                                        HD * h0:HD * h0 + 2 * HD],
                              in_=otile[:])
                    return flush_out
                return av

            for pair in range(4):
                for qb in range(2):
                    units.append(mk_unit(pair, qb))
            return units

        # emission: batch 0 DMAs + conv-k prefix up front, then attn(b)
        # interleaved with [v path of b, DMAs + prefix of b+1] as fillers.
        d0, a0, pending = prep_chunks(0)
        for ch in d0 + a0:
            ch()
        flush = None
        av_q = []  # AVs awaiting emission, deferred two units
        for b in range(BPC):
            units = attn_units(b)
            urgent = pending  # this batch's v path: needed by its own AVs
            if b + 1 < BPC:
                d1, a1, b1 = prep_chunks(b + 1)
                lazy = d1 + a1
                pending = b1
            else:
                lazy = []
                pending = []
            n_u = len(units)
            for i, unit in enumerate(units):
                fl = []
                fl += urgent[i * len(urgent) // URG_U:
                             (i + 1) * len(urgent) // URG_U]
                if i >= URG_U:
                    j = i - URG_U
                    fl += lazy[j * len(lazy) // (n_u - URG_U):
                               (j + 1) * len(lazy) // (n_u - URG_U)]
                av = unit(fillers=fl, pre_out=flush, pre_av=None)
                av_q.append(av)
                if len(av_q) > AV_DEFER:
                    flush = av_q.pop(0)()
                else:
                    flush = None
        while av_q:
            fl = av_q.pop(0)()
            if flush is not None:
                flush()
            flush = fl
        if flush is not None:
            flush()
            flush = None

    return nc


def _get_nc():
    if "nc" not in _CACHE:
        import concourse.bass as bass
        import concourse.tile as tile
        from concourse import bacc, mybir

        nc = bacc.Bacc("TRN2", target_bir_lowering=False, debug=False)
        _build(nc, tile, bass, mybir)
        nc.compile()
        _CACHE["nc"] = nc
    return _CACHE["nc"]


def kernel(**inputs):
    q = np.ascontiguousarray(
        np.asarray(inputs["query"], dtype=np.float32).reshape(B, S, C))
    k = np.ascontiguousarray(
        np.asarray(inputs["key_in"], dtype=np.float32).reshape(B, S, C))
    v = np.ascontiguousarray(
        np.asarray(inputs["value"], dtype=np.float32).reshape(B, S, C))
    ck = np.ascontiguousarray(
        np.asarray(inputs["conv_kernel"], dtype=np.float32).reshape(
            KSZ * KSZ, C))
    cb = np.ascontiguousarray(
        np.asarray(inputs["conv_bias"], dtype=np.float32).reshape(C))

    in_maps = []
    for i in range(NCORES):
        lo, hi = i * BPC, (i + 1) * BPC
        in_maps.append({
            "query": np.ascontiguousarray(q[lo:hi]),
            "key_in": np.ascontiguousarray(k[lo:hi]),
            "value": np.ascontiguousarray(v[lo:hi]),
            "conv_kernel": ck,
            "conv_bias": cb,
        })

    from concourse.bass_utils import run_bass_kernel_spmd

    nc = _get_nc()
    res = run_bass_kernel_spmd(
        nc, in_maps, core_ids=list(range(NCORES)),
        **_CACHE.get("run_kwargs", {}),
    )
    _CACHE["last_result"] = res
    out = np.concatenate([r["out"] for r in res.results], axis=0)
    return out.reshape(B, H, W, C)
